# revision 50
# baseline (speedup 1.0000x reference)
"""MultiHeadCrossAttention Trainium2 kernel (8-core SPMD, query-parallel).

Sharding: core c handles batch b=c//4, query rows [1024*(c%4), +1024), all 8
heads.  Each core returns a disjoint [256, 1024] slice of out^T for its batch;
the host gather is a pure concat + transpose.

On-device layout is fully transposed ([channel, position]), matching the raw
[B, C, H, W] input layout, so no transposes are needed anywhere:
  q^T/k^T : [d, pos]   via  lhsT=W^T chunk [c,32|128], rhs=x^T chunk [c, pos]
  scores^T: [kpos, q]  via  lhsT=k^T [32,128] row-tiled 4x, rhs=q^T [32,512]
  exp     : split between ACT (activation Exp, PSUM->SBUF bf16) and DVE
            (Schraudolph: tensor_scalar fp32*A+B -> int16, bitcast bf16;
            verified round-to-nearest + saturation on HW, ~3% rel err that
            cancels through the softmax normalization)
  attn@v  : lhsT=[v|1] [128,33], rhs=p^T [128,512]; ones column yields
            softmax denominators in row 32 of the accumulating matmul
  norm    : po drained (with sums row) to fp32 xm; sums DMA-gathered to
            [16,64], reciprocal_approx_fast at base 0, DMA to [1,NQ] row,
            gpsimd partition-broadcast, one DVE mul -> xf bf16
  final   : xf DMA-stacked 4-heads-per-[128,NQ] tile; out^T accumulated in
            PSUM over 4 matmuls per (dc,qb): 2 xf-stack chunks + 2 tgt^T
            chunks (the residual is folded into the projection)

Emission order software-pipelines head h+1's projections and head h-1's
normalization under head h's attention so ACT/DVE never stall on DMA or
gpsimd latency (both engines have 8-deep strict-FIFO queues).
"""

import numpy as np
import ml_dtypes

B, C, N, HEADS, D = 2, 256, 4096, 8, 32
NQ = 1024          # queries per core
NCORES = 8
CC = C // 128      # contraction chunks (2)

BF16 = ml_dtypes.bfloat16

# Schraudolph exp for bf16 bit layout (int16 add path), verified on HW:
# int16(round(x*EXPA + EXPB)).view(bf16) ~= exp(x), max rel err 3.3%
EXPA = float(128.0 / np.log(2.0))
EXPB = float(128.0 * (127.0 - 0.0436775))

_cached = {}
# dve_tiles: per attn_unit, set of (round, tile_idx) whose exp runs on the
# DVE instead of ACT.  16 tiles/unit; ACT:DVE defaults 10:6.
CFG = {
    "dve_tiles": {(1, 1), (2, 1), (3, 1), (5, 1), (6, 1), (7, 1)},
    "kq_drain_act": False,   # kT/qT projection drains on ACT instead of DVE
    "v_drain_act": False,    # vproj drains on ACT instead of DVE
}


def _build_nc():
    import concourse.bass as bass
    import concourse.bacc as bacc
    import concourse.tile as tile
    import concourse.mybir as mybir
    from contextlib import ExitStack

    fp32 = mybir.dt.float32
    bf16 = mybir.dt.bfloat16
    i16 = mybir.dt.int16
    Exp = mybir.ActivationFunctionType.Exp
    Mult = mybir.AluOpType.mult
    Add = mybir.AluOpType.add

    nc = bacc.Bacc("TRN2", target_bir_lowering=False, debug=False,
                   num_devices=NCORES)

    src_d = nc.dram_tensor("src_bf", [C, N], bf16, kind="ExternalInput")
    tgt_d = nc.dram_tensor("tgt_bf", [C, NQ], bf16, kind="ExternalInput")
    wq4_d = nc.dram_tensor("wq4", [C, HEADS * 128], bf16, kind="ExternalInput")
    wk_d = nc.dram_tensor("wkT", [C, C], bf16, kind="ExternalInput")
    wv_d = nc.dram_tensor("wvT", [C, C], bf16, kind="ExternalInput")
    wo_d = nc.dram_tensor("woT", [C, C], bf16, kind="ExternalInput")
    y_d = nc.dram_tensor("yT", [C, NQ], fp32, kind="ExternalOutput")
    if CFG.get("debug"):
        dbg_xm_d = nc.dram_tensor("dbg_xm", [32, NQ], fp32,
                                  kind="ExternalOutput")
        dbg_ru_d = nc.dram_tensor("dbg_ru", [16, 64], fp32,
                                  kind="ExternalOutput")
        dbg_rbs_d = nc.dram_tensor("dbg_rbs", [32, NQ], fp32,
                                   kind="ExternalOutput")
        dbg_xf_d = nc.dram_tensor("dbg_xf", [32, NQ], fp32,
                                  kind="ExternalOutput")

    with tile.TileContext(nc) as tc, ExitStack() as ctx:
        konst = ctx.enter_context(tc.tile_pool(name="konst", bufs=1))
        work = ctx.enter_context(tc.tile_pool(name="work", bufs=1))
        p_pool = ctx.enter_context(tc.tile_pool(name="p", bufs=16))
        sm_pool = ctx.enter_context(tc.tile_pool(name="sm", bufs=2))
        xb_pool = ctx.enter_context(tc.tile_pool(name="xb", bufs=2))
        # PSUM budget (8 banks): ps tiles [128,1024] (2 banks each) x3,
        # po tiles [128,512] (1 bank) x2 shared with projection psum
        ps_pool = ctx.enter_context(tc.tile_pool(name="ps", bufs=3, space="PSUM"))
        po_pool = ctx.enter_context(tc.tile_pool(name="po", bufs=2, space="PSUM"))
        pj_pool = po_pool

        # ---- load inputs (ordered so kproj/vproj deps land first; the DMA
        # stream is HBM-bound ~15us and overlaps the first attention rounds)
        src_sb = konst.tile([128, CC * N], bf16, tag="src")
        tgt_sb = konst.tile([128, CC * NQ], bf16, tag="tgt")
        wq4_sb = konst.tile([128, CC * HEADS * 128], bf16, tag="wq4")
        wk_sb = konst.tile([128, CC * C], bf16, tag="wk")
        wv_sb = konst.tile([128, CC * C], bf16, tag="wv")
        wo_sb = konst.tile([128, CC * C], bf16, tag="wo")

        def dma_w(w_sb, w_d, eng=None):
            for cc in range(CC):
                (eng or nc.sync).dma_start(w_sb[:, cc * C:(cc + 1) * C],
                                           w_d.ap()[128 * cc:128 * (cc + 1), :])

        def dma_src_half(half, eng=None):
            for cc in range(CC):
                (eng or nc.sync).dma_start(
                    src_sb[:, cc * N + 2048 * half: cc * N + 2048 * (half + 1)],
                    src_d.ap()[128 * cc:128 * (cc + 1),
                               2048 * half:2048 * (half + 1)])

        dma_w(wk_sb, wk_d)
        dma_src_half(0)
        for cc in range(CC):
            nc.sync.dma_start(wq4_sb[:, cc * 1024:(cc + 1) * 1024],
                              wq4_d.ap()[128 * cc:128 * (cc + 1), :])
        for cc in range(CC):
            nc.sync.dma_start(tgt_sb[:, cc * NQ:(cc + 1) * NQ],
                              tgt_d.ap()[128 * cc:128 * (cc + 1), :])
        dma_w(wv_sb, wv_d)
        dma_src_half(1)
        dma_w(wo_sb, wo_d)

        # ---- persistent tiles ---------------------------------------------
        kT = [konst.tile([128, 1024], bf16, tag=f"kT{h}", name=f"kT{h}")
              for h in range(HEADS)]
        qT = [konst.tile([128, NQ], bf16, tag=f"qT{h}", name=f"qT{h}")
              for h in range(HEADS)]
        v_sb = konst.tile([128, HEADS * 33 * 32], bf16, tag="v")
        for h in range(HEADS):
            ones_ap = v_sb[:].rearrange("p (h k c) -> p h k c", h=HEADS, k=32)[
                :, h, :, 32:33]
            nc.gpsimd.memset(ones_ap, 1.0)
        # K=1 broadcast weights for the rbs partition-broadcast matmul
        ones1_sb = konst.tile([1, 32], bf16, tag="ones1")
        nc.gpsimd.memset(ones1_sb[:], 1.0)
        # xm: merged attn@v output + sums row, fp32, per head
        xm = [work.tile([33, NQ], fp32, tag=f"xm{h}", name=f"xm{h}")
              for h in range(HEADS)]
        # per-head softmax sums [16,64] at base 0: su[p,f] = sums[64p+f]
        su = [work.tile([16, 64], fp32, tag=f"su{h}", name=f"su{h}")
              for h in range(HEADS)]
        ru = [work.tile([16, 64], fp32, tag=f"ru{h}", name=f"ru{h}")
              for h in range(HEADS)]
        # bf16 copies of the tail heads' reciprocals (bf16 broadcast matmul)
        rub = {h: work.tile([16, 64], bf16, tag=f"rub{h}", name=f"rub{h}")
               for h in (6, 7)}
        # normalized per-head output, bf16
        xf = [work.tile([32, NQ], bf16, tag=f"xf{h}", name=f"xf{h}")
              for h in range(HEADS)]
        # 4-head stacks for the output projection
        xfs = [konst.tile([128, NQ], bf16, tag=f"xfs{t}", name=f"xfs{t}")
               for t in range(CC)]

        kq_eng = nc.scalar if CFG["kq_drain_act"] else nc.vector
        v_eng = nc.scalar if CFG["v_drain_act"] else nc.vector

        def drain(eng, dst, src):
            if eng is nc.scalar:
                nc.scalar.activation(dst, src,
                                     mybir.ActivationFunctionType.Copy)
            else:
                nc.vector.tensor_copy(dst, src)

        v_done = set()

        def vproj(kc):
            if kc in v_done:
                return
            v_done.add(kc)
            ps = pj_pool.tile([128, 512], fp32, tag="po", name=f"psv{kc}")
            for cc in range(CC):
                nc.tensor.matmul(
                    ps[:, 0:256],
                    lhsT=src_sb[:, cc * N + 128 * kc: cc * N + 128 * kc + 128],
                    rhs=wv_sb[:, cc * C:(cc + 1) * C],
                    start=(cc == 0), stop=(cc == CC - 1),
                    tile_position=(0, 0))
            dest = v_sb[:].rearrange("p (h k c) -> p h k c", h=HEADS, k=32)[
                :, :, kc, 0:32]
            drain(v_eng, dest, ps[:, 0:256])

        def kqproj_steps(h):
            # k^T folded: strip g (partitions 32g..) holds kpos block b=4jj+g
            # at cols [512jj, +512); kc for 128-col slice m: 16*(m//4)+4g+(m%4)
            steps = []

            def k_step(jj, cc, ps_box):
                def run():
                    if cc == 0:
                        ps_box.append(pj_pool.tile(
                            [128, 512], fp32, tag="po", name=f"psk{h}_{jj}"))
                    ps = ps_box[0]
                    for g in range(4):
                        blk = 4 * jj + g
                        nc.tensor.matmul(
                            ps[32 * g:32 * g + 32, 0:512],
                            lhsT=wk_sb[:, cc * C + 32 * h: cc * C + 32 * h + 32],
                            rhs=src_sb[:, cc * N + 512 * blk: cc * N + 512 * blk + 512],
                            start=(cc == 0), stop=(cc == CC - 1),
                            # 4 col-strip groups share this bank; HW
                            # has_written is per-element, the sim's
                            # zero-region check is coarser
                            skip_group_check=True,
                            tile_position=(0, 32 * g))
                    if cc == CC - 1:
                        drain(kq_eng, kT[h][:, 512 * jj:512 * jj + 512],
                              ps[:, 0:512])
                return run

            def q_step(qb):
                def run():
                    ps = pj_pool.tile([128, 512], fp32, tag="po",
                                      name=f"psq{h}_{qb}")
                    for cc in range(CC):
                        nc.tensor.matmul(
                            ps[:, 0:512],
                            lhsT=wq4_sb[:, cc * 1024 + 128 * h: cc * 1024 + 128 * h + 128],
                            rhs=tgt_sb[:, cc * NQ + 512 * qb: cc * NQ + 512 * qb + 512],
                            start=(cc == 0), stop=(cc == CC - 1),
                            tile_position=(0, 0))
                    drain(kq_eng, qT[h][:, 512 * qb:512 * qb + 512],
                          ps[:, 0:512])
                return run

            for jj in range(2):
                box = []
                for cc in range(CC):
                    steps.append(k_step(jj, cc, box))
            for qb in range(NQ // 512):
                steps.append(q_step(qb))
            return steps

        def kqproj(h):
            for st in kqproj_steps(h):
                st()

        strips = (0, 1, 2, 3)
        po_tiles = {}
        merges = []

        def merge_pop():
            """Fold the pending unit's B half into its A half: one DVE
            tensor_tensor (PSUM A + hopped-B SBUF -> xm fp32, incl. the sums
            row), then DMA the merged sums into su.  Releases the po bank."""
            h, qb, po, bhop = merges.pop(0)
            nc.vector.tensor_add(xm[h][:, 512 * qb:512 * qb + 512],
                                 po[0:33, 0:512], bhop[0:33, :])
            nc.sync.dma_start(su[h][8 * qb:8 * qb + 8, 0:64],
                              xm[h][32:33, 512 * qb:512 * qb + 512])
            del po_tiles[(h, qb)]

        def attnv(h, qb, m, pbs):
            """attn@v quad for (unit, round m), col-tiled 2x: strips 0/2
            accumulate at partitions 0-32 (A), strips 1/3 at 64-96 (B) of the
            same bank-columns, so the two chains stream concurrently.  po is
            allocated lazily at the unit's first (lagged) quad."""
            if (h, qb) not in po_tiles:
                po_tiles[(h, qb)] = po_pool.tile(
                    [128, 512], fp32, tag="po", name=f"po{h}_{qb}")
            po = po_tiles[(h, qb)]
            for gi, g in enumerate(strips):
                kc = 16 * (m // 4) + 4 * g + (m % 4)
                co = 64 * (gi % 2)
                nc.tensor.matmul(
                    po[co:co + 33, 0:512],
                    lhsT=v_sb[:, 1056 * h + 33 * kc: 1056 * h + 33 * kc + 33],
                    rhs=pbs[gi // 2][:, 512 * (gi % 2):512 * (gi % 2) + 512],
                    start=(m == 0 and gi <= 1), stop=(m == 7 and gi >= 2),
                    skip_group_check=True,
                    tile_position=(0, co))
            if m == 7:
                # B half drains on ACT and hops to partitions 0-32 by DMA;
                # the A half stays in PSUM until merge() adds them in a
                # single DVE tensor_tensor (PSUM + SBUF -> xm), two rounds
                # later.  gpsimd runs ONLY partition_broadcast (mixing in
                # other gpsimd ucode would force 6us IRAM reloads per head).
                stgb = xb_pool.tile([97, 512], fp32, tag="stgb",
                                    name=f"sb{h}_{qb}")
                bhop = xb_pool.tile([33, 512], fp32, tag="bhop",
                                    name=f"bh{h}_{qb}")
                nc.vector.tensor_copy(stgb[64:97, :], po[64:97, 0:512])
                nc.sync.dma_start(bhop[0:33, :], stgb[64:97, :])
                merges.append((h, qb, po, bhop))

        def score_exp(h, qb, ri, feed):
            """scores quad + exp pair for (unit, round ri); feed maps round
            index -> closure (pinned so deps are emitted in order)."""
            m = ri
            st = feed.pop(ri, None)
            if st is not None:
                st()
            pss = [ps_pool.tile([128, 1024], fp32, tag="ps",
                                name=f"ps{h}_{qb}_{ri}_{i}")
                   for i in range(2)]
            for gi, g in enumerate(strips):
                nc.tensor.matmul(
                    pss[gi // 2][:, 512 * (gi % 2):512 * (gi % 2) + 512],
                    lhsT=kT[h][32 * g:32 * g + 32, 128 * m:128 * m + 128],
                    rhs=qT[h][32 * g:32 * g + 32, 512 * qb:512 * qb + 512],
                    start=True, stop=True,
                    tile_position=(32 * g, 0))
            pbs = []
            for i in range(2):
                p_sb = p_pool.tile([128, 1024], bf16, tag="p",
                                   name=f"p{h}_{qb}_{ri}_{i}")
                if (ri, i) in CFG["dve_tiles"]:
                    nc.vector.tensor_scalar(
                        p_sb[:].bitcast(i16), pss[i][:, 0:1024],
                        EXPA, EXPB, Mult, Add)
                else:
                    nc.scalar.activation(p_sb[:], pss[i][:, 0:1024], Exp)
                pbs.append(p_sb)
            # vproj for this round's chunks emits after the exps so the exp
            # stream starts as early as possible during the first unit
            if h == 0 and qb == 0:
                for g in strips:
                    vproj(16 * (m // 4) + 4 * g + (m % 4))
            if ri == 3 and merges:
                merge_pop()
            return pbs

        def norm_steps(h):
            """Normalization closures for head h, to be fed under later
            units.  Steady-state heads broadcast 1/denominator by doubling
            DMAs and multiply on gpsimd (exp engines stay free); the last two
            heads use a K=1 PE-matmul broadcast + DVE multiply, which has a
            much shorter serial chain, to compress the kernel tail."""
            tail = h >= 6
            rrow = sm_pool.tile([1, NQ], bf16 if tail else fp32, tag="rrow",
                                name=f"rr{h}")

            def stack():
                t, r = h // 4, 32 * (h % 4)
                nc.sync.dma_start(xfs[t][r:r + 32, :], xf[h][:])

            if not tail:
                rbs = sm_pool.tile([32, NQ], fp32, tag="rbs", name=f"rb{h}")

                def s_recip():
                    nc.vector.reciprocal_approx_fast(ru[h][:], su[h][:])
                    nc.sync.dma_start(rrow[:], ru[h][:])
                    nc.sync.dma_start(rbs[0:1, :], rrow[:])
                    for w in (1, 2, 4, 8, 16):
                        nc.sync.dma_start(rbs[w:2 * w, :], rbs[0:w, :])

                def s_mul():
                    nc.gpsimd.tensor_mul(xf[h][:], xm[h][0:32, :], rbs[:])
                    stack()

                return [s_recip, s_mul, lambda: None]

            def s_recip():
                nc.vector.reciprocal_approx_fast(ru[h][:], su[h][:])
                nc.vector.tensor_copy(rub[h][:], ru[h][:])
                nc.sync.dma_start(rrow[:], rub[h][:])

            def s_mul(q2):
                rps = pj_pool.tile([128, 512], fp32, tag="po",
                                   name=f"rb{h}_{q2}")
                nc.tensor.matmul(
                    rps[0:32, 0:512], lhsT=ones1_sb[0:1, 0:32],
                    rhs=rrow[0:1, 512 * q2:512 * q2 + 512],
                    start=True, stop=True, tile_position=(0, 0))
                nc.vector.tensor_mul(
                    xf[h][:, 512 * q2:512 * q2 + 512],
                    xm[h][0:32, 512 * q2:512 * q2 + 512],
                    rps[0:32, 0:512])
                if q2 == 1:
                    stack()

            return [s_recip, lambda: s_mul(0), lambda: s_mul(1)]

        def out_proj():
            for dc in range(CC):
                for qb in range(NQ // 512):
                    ps = pj_pool.tile([128, 512], fp32, tag="po",
                                      name=f"py{dc}_{qb}")
                    # tgt chunks first, xfs last: the first MMs can run
                    # while the final heads' normalization still completes
                    rhss = [(cc, tgt_sb[:, cc * NQ + 512 * qb:
                                        cc * NQ + 512 * qb + 512])
                            for cc in range(CC)]
                    rhss += [(cc, xfs[cc][:, 512 * qb:512 * qb + 512])
                             for cc in range(CC)]
                    for n_mm, (cc, rhs) in enumerate(rhss):
                        nc.tensor.matmul(
                            ps[:, 0:512],
                            lhsT=wo_sb[:, cc * C + 128 * dc: cc * C + 128 * dc + 128],
                            rhs=rhs,
                            start=(n_mm == 0), stop=(n_mm == 3),
                            tile_position=(0, 0))
                    yt = xb_pool.tile([128, 512], fp32, tag="yt",
                                      name=f"yt{dc}_{qb}")
                    nc.vector.tensor_copy(yt[:], ps[:, 0:512])
                    nc.sync.dma_start(
                        y_d.ap()[128 * dc:128 * (dc + 1),
                                 512 * qb:512 * qb + 512], yt[:])

        # ---- emission: one flat global round stream across all 16 units;
        # attn@v lags scores/exp by ATTNV_LAG rounds so it never waits on
        # exp and unit boundaries don't drain the pipeline -------------------
        ATTNV_LAG = 3
        kqproj(0)
        units = [(h, qb) for h in range(HEADS) for qb in range(NQ // 512)]
        # feed maps round index -> closure.  kqproj steps early; norm steps
        # pinned at round >= 2 so they trail the previous unit's drain/su
        # gather, which the lag-2 attn@v emits at rounds 0-1 of this unit.
        feeds = {}
        for h in range(HEADS):
            steps = kqproj_steps(h + 1) if h + 1 < HEADS else []
            nsteps = norm_steps(h - 1) if h >= 1 else []
            feeds[(h, 0)] = {}
            feeds[(h, 1)] = {}
            if steps:
                # k halves at rounds {0,1}/{4,5}; q steps on the all-ACT
                # rounds 0/4 so they never queue behind a DVE exp
                feeds[(h, 0)][0] = steps[0]
                feeds[(h, 0)][1] = steps[1]
                feeds[(h, 0)][4] = steps[2]
                feeds[(h, 0)][5] = steps[3]
                feeds[(h, 1)][0] = steps[4]
                feeds[(h, 1)][4] = steps[5]
            if nsteps:
                feeds[(h, 0)][6] = nsteps[0]
                feeds[(h, 1)][5] = nsteps[1]
                feeds[(h, 1)][7] = nsteps[2]
        pend = []
        for ui, (h, qb) in enumerate(units):
            feed = feeds[(h, qb)]
            # unit 0 runs at a deeper lag so its vproj PE work doesn't
            # throttle the exp stream at startup; drained 2 quads/round after
            lag = 6 if ui == 0 else ATTNV_LAG
            for ri in range(8):
                pbs = score_exp(h, qb, ri, feed)
                pend.append((h, qb, ri, pbs))
                for _ in range(min(max(len(pend) - lag, 0), 2)):
                    attnv(*pend.pop(0))
            assert not feed
        for args in pend:
            attnv(*args)
        while merges:
            merge_pop()
        for st in norm_steps(HEADS - 1):
            st()
        out_proj()

    nc.compile()
    return nc


def _prep_core_inputs(core, tgt, src, Wq, Wk, Wv, Wo):
    b, qoff = core // 4, NQ * (core % 4)
    srcT = src[b].reshape(C, N)
    tgtT = tgt[b].reshape(C, N)[:, qoff:qoff + NQ]
    scale = 1.0 / np.sqrt(np.float32(D))
    wqT = (Wq * scale).T.astype(BF16)
    wq4 = np.empty((C, HEADS * 128), dtype=BF16)
    for h in range(HEADS):
        wq4[:, 128 * h:128 * (h + 1)] = np.tile(wqT[:, 32 * h:32 * h + 32],
                                                (1, 4))
    return {
        "src_bf": np.ascontiguousarray(srcT).astype(BF16),
        "tgt_bf": np.ascontiguousarray(tgtT).astype(BF16),
        "wq4": wq4,
        "wkT": np.ascontiguousarray(Wk.T).astype(BF16),
        "wvT": np.ascontiguousarray(Wv.T).astype(BF16),
        "woT": np.ascontiguousarray(Wo.T).astype(BF16),
    }


def kernel(tgt, src, Wq, Wk, Wv, Wo, _want_results=False):
    from concourse.bass_utils import run_bass_kernel_spmd

    tgt = np.asarray(tgt, dtype=np.float32)
    src = np.asarray(src, dtype=np.float32)
    Wq = np.asarray(Wq, dtype=np.float32)
    Wk = np.asarray(Wk, dtype=np.float32)
    Wv = np.asarray(Wv, dtype=np.float32)
    Wo = np.asarray(Wo, dtype=np.float32)

    if "nc" not in _cached:
        _cached["nc"] = _build_nc()
    nc = _cached["nc"]

    in_maps = [_prep_core_inputs(c, tgt, src, Wq, Wk, Wv, Wo)
               for c in range(NCORES)]
    res = run_bass_kernel_spmd(nc, in_maps, core_ids=list(range(NCORES)))

    out = np.empty((B, N, C), dtype=np.float32)
    for c in range(NCORES):
        b, qoff = c // 4, NQ * (c % 4)
        out[b, qoff:qoff + NQ, :] = res.results[c]["yT"].T
    if _want_results:
        return out, res
    return out


# revision 52
# speedup vs baseline: 1.1108x; 1.1108x over previous
"""MultiHeadCrossAttention Trainium2 kernel (8-core SPMD, query-parallel).

Sharding: core c handles batch b=c//4, query rows [1024*(c%4), +1024), all 8
heads.  Each core returns a disjoint [256, 1024] slice of out^T for its batch;
the host gather is a pure concat + transpose.

On-device layout is fully transposed ([channel, position]), matching the raw
[B, C, H, W] input layout, so no transposes are needed anywhere:
  q^T/k^T : [d, pos]   via  lhsT=W^T chunk [c,32|128], rhs=x^T chunk [c, pos]
  scores^T: [kpos, q]  via  lhsT=k^T [32,128] row-tiled 4x, rhs=q^T [32,512]
  exp     : split between ACT (activation Exp, PSUM->SBUF bf16) and DVE
            (Schraudolph: tensor_scalar fp32*A+B -> int16, bitcast bf16;
            verified round-to-nearest + saturation on HW, ~3% rel err that
            cancels through the softmax normalization)
  attn@v  : lhsT=[v|1] [128,33], rhs=p^T [128,512]; ones column yields
            softmax denominators in row 32 of the accumulating matmul
  norm    : po drained (with sums row) to fp32 xm; sums DMA-gathered to
            [16,64], reciprocal_approx_fast at base 0, DMA to [1,NQ] row,
            gpsimd partition-broadcast, one DVE mul -> xf bf16
  final   : xf DMA-stacked 4-heads-per-[128,NQ] tile; out^T accumulated in
            PSUM over 4 matmuls per (dc,qb): 2 xf-stack chunks + 2 tgt^T
            chunks (the residual is folded into the projection)

Emission order software-pipelines head h+1's projections and head h-1's
normalization under head h's attention so ACT/DVE never stall on DMA or
gpsimd latency (both engines have 8-deep strict-FIFO queues).
"""

import numpy as np
import ml_dtypes

B, C, N, HEADS, D = 2, 256, 4096, 8, 32
NQ = 1024          # queries per core
NCORES = 8
CC = C // 128      # contraction chunks (2)

BF16 = ml_dtypes.bfloat16

# Schraudolph exp for bf16 bit layout (int16 add path), verified on HW:
# int16(round(x*EXPA + EXPB)).view(bf16) ~= exp(x), max rel err 3.3%
EXPA = float(128.0 / np.log(2.0))
EXPB = float(128.0 * (127.0 - 0.0436775))

_cached = {}
# dve_tiles: per attn_unit, set of (round, tile_idx) whose exp runs on the
# DVE instead of ACT.  16 tiles/unit; ACT:DVE defaults 10:6.
CFG = {
    "dve_tiles": {(1, 1), (2, 1), (3, 1), (5, 1), (6, 1), (7, 1)},
    "kq_drain_act": False,   # kT/qT projection drains on ACT instead of DVE
    "v_drain_act": False,    # vproj drains on ACT instead of DVE
}


def _build_nc():
    import concourse.bass as bass
    import concourse.bacc as bacc
    import concourse.tile as tile
    import concourse.mybir as mybir
    from contextlib import ExitStack

    fp32 = mybir.dt.float32
    bf16 = mybir.dt.bfloat16
    i16 = mybir.dt.int16
    Exp = mybir.ActivationFunctionType.Exp
    Mult = mybir.AluOpType.mult
    Add = mybir.AluOpType.add

    nc = bacc.Bacc("TRN2", target_bir_lowering=False, debug=False,
                   num_devices=NCORES)

    src_d = nc.dram_tensor("src_bf", [C, N], bf16, kind="ExternalInput")
    tgt_d = nc.dram_tensor("tgt_bf", [C, NQ], bf16, kind="ExternalInput")
    wq4_d = nc.dram_tensor("wq4", [C, HEADS * 128], bf16, kind="ExternalInput")
    wk_d = nc.dram_tensor("wkT", [C, C], bf16, kind="ExternalInput")
    wv_d = nc.dram_tensor("wvT", [C, C], bf16, kind="ExternalInput")
    wo_d = nc.dram_tensor("woT", [C, C], bf16, kind="ExternalInput")
    y_d = nc.dram_tensor("yT", [C, NQ], fp32, kind="ExternalOutput")
    if CFG.get("debug"):
        dbg_xm_d = nc.dram_tensor("dbg_xm", [32, NQ], fp32,
                                  kind="ExternalOutput")
        dbg_ru_d = nc.dram_tensor("dbg_ru", [16, 64], fp32,
                                  kind="ExternalOutput")
        dbg_rbs_d = nc.dram_tensor("dbg_rbs", [32, NQ], fp32,
                                   kind="ExternalOutput")
        dbg_xf_d = nc.dram_tensor("dbg_xf", [32, NQ], fp32,
                                  kind="ExternalOutput")

    with tile.TileContext(nc) as tc, ExitStack() as ctx:
        konst = ctx.enter_context(tc.tile_pool(name="konst", bufs=1))
        work = ctx.enter_context(tc.tile_pool(name="work", bufs=1))
        p_pool = ctx.enter_context(tc.tile_pool(name="p", bufs=16))
        sm_pool = ctx.enter_context(tc.tile_pool(name="sm", bufs=2))
        xb_pool = ctx.enter_context(tc.tile_pool(name="xb", bufs=2))
        # PSUM budget (8 banks): ps tiles [128,1024] (2 banks each) x3,
        # po tiles [128,512] (1 bank) x2 shared with projection psum
        ps_pool = ctx.enter_context(tc.tile_pool(name="ps", bufs=3, space="PSUM"))
        po_pool = ctx.enter_context(tc.tile_pool(name="po", bufs=2, space="PSUM"))
        pj_pool = po_pool

        # ---- load inputs (ordered so kproj/vproj deps land first; the DMA
        # stream is HBM-bound ~15us and overlaps the first attention rounds)
        src_sb = konst.tile([128, CC * N], bf16, tag="src")
        tgt_sb = konst.tile([128, CC * NQ], bf16, tag="tgt")
        wq4_sb = konst.tile([128, CC * HEADS * 128], bf16, tag="wq4")
        wk_sb = konst.tile([128, CC * C], bf16, tag="wk")
        wv_sb = konst.tile([128, CC * C], bf16, tag="wv")
        wo_sb = konst.tile([128, CC * C], bf16, tag="wo")

        def dma_w(w_sb, w_d, eng=None):
            for cc in range(CC):
                (eng or nc.sync).dma_start(w_sb[:, cc * C:(cc + 1) * C],
                                           w_d.ap()[128 * cc:128 * (cc + 1), :])

        def dma_src_half(half, eng=None):
            for cc in range(CC):
                (eng or nc.sync).dma_start(
                    src_sb[:, cc * N + 2048 * half: cc * N + 2048 * (half + 1)],
                    src_d.ap()[128 * cc:128 * (cc + 1),
                               2048 * half:2048 * (half + 1)])

        dma_w(wk_sb, wk_d)
        dma_src_half(0)
        for cc in range(CC):
            nc.sync.dma_start(wq4_sb[:, cc * 1024:(cc + 1) * 1024],
                              wq4_d.ap()[128 * cc:128 * (cc + 1), :])
        for cc in range(CC):
            nc.sync.dma_start(tgt_sb[:, cc * NQ:(cc + 1) * NQ],
                              tgt_d.ap()[128 * cc:128 * (cc + 1), :])
        dma_w(wv_sb, wv_d)
        dma_src_half(1)
        dma_w(wo_sb, wo_d)

        # ---- persistent tiles ---------------------------------------------
        kT = [konst.tile([128, 1024], bf16, tag=f"kT{h}", name=f"kT{h}")
              for h in range(HEADS)]
        qT = [konst.tile([128, NQ], bf16, tag=f"qT{h}", name=f"qT{h}")
              for h in range(HEADS)]
        v_sb = konst.tile([128, HEADS * 33 * 32], bf16, tag="v")
        for h in range(HEADS):
            ones_ap = v_sb[:].rearrange("p (h k c) -> p h k c", h=HEADS, k=32)[
                :, h, :, 32:33]
            nc.gpsimd.memset(ones_ap, 1.0)
        # K=1 broadcast weights for the rbs partition-broadcast matmul
        ones1_sb = konst.tile([1, 32], bf16, tag="ones1")
        nc.gpsimd.memset(ones1_sb[:], 1.0)
        # xm: merged attn@v output + sums row, fp32, per head
        xm = [work.tile([33, NQ], fp32, tag=f"xm{h}", name=f"xm{h}")
              for h in range(HEADS)]
        # per-head softmax sums [16,64] at base 0: su[p,f] = sums[64p+f]
        su = [work.tile([16, 64], fp32, tag=f"su{h}", name=f"su{h}")
              for h in range(HEADS)]
        ru = [work.tile([16, 64], fp32, tag=f"ru{h}", name=f"ru{h}")
              for h in range(HEADS)]
        # bf16 copies of the tail heads' reciprocals (bf16 broadcast matmul)
        rub = {h: work.tile([16, 64], bf16, tag=f"rub{h}", name=f"rub{h}")
               for h in (6, 7)}
        # normalized per-head output, bf16
        xf = [work.tile([32, NQ], bf16, tag=f"xf{h}", name=f"xf{h}")
              for h in range(HEADS)]
        # 4-head stacks for the output projection
        xfs = [konst.tile([128, NQ], bf16, tag=f"xfs{t}", name=f"xfs{t}")
               for t in range(CC)]

        kq_eng = nc.scalar if CFG["kq_drain_act"] else nc.vector
        v_eng = nc.scalar if CFG["v_drain_act"] else nc.vector

        def drain(eng, dst, src):
            if eng is nc.scalar:
                nc.scalar.activation(dst, src,
                                     mybir.ActivationFunctionType.Copy)
            else:
                nc.vector.tensor_copy(dst, src)

        v_done = set()

        def vproj(kc):
            if kc in v_done:
                return
            v_done.add(kc)
            ps = pj_pool.tile([128, 512], fp32, tag="po", name=f"psv{kc}")
            for cc in range(CC):
                nc.tensor.matmul(
                    ps[:, 0:256],
                    lhsT=src_sb[:, cc * N + 128 * kc: cc * N + 128 * kc + 128],
                    rhs=wv_sb[:, cc * C:(cc + 1) * C],
                    start=(cc == 0), stop=(cc == CC - 1),
                    tile_position=(0, 0))
            dest = v_sb[:].rearrange("p (h k c) -> p h k c", h=HEADS, k=32)[
                :, :, kc, 0:32]
            drain(v_eng, dest, ps[:, 0:256])

        def kqproj_steps(h):
            # k^T folded: strip g (partitions 32g..) holds kpos block b=4jj+g
            # at cols [512jj, +512); kc for 128-col slice m: 16*(m//4)+4g+(m%4)
            steps = []

            def k_step(jj, cc, ps_box):
                def run():
                    if cc == 0:
                        ps_box.append(pj_pool.tile(
                            [128, 512], fp32, tag="po", name=f"psk{h}_{jj}"))
                    ps = ps_box[0]
                    for g in range(4):
                        blk = 4 * jj + g
                        nc.tensor.matmul(
                            ps[32 * g:32 * g + 32, 0:512],
                            lhsT=wk_sb[:, cc * C + 32 * h: cc * C + 32 * h + 32],
                            rhs=src_sb[:, cc * N + 512 * blk: cc * N + 512 * blk + 512],
                            start=(cc == 0), stop=(cc == CC - 1),
                            # 4 col-strip groups share this bank; HW
                            # has_written is per-element, the sim's
                            # zero-region check is coarser
                            skip_group_check=True,
                            tile_position=(0, 32 * g))
                    if cc == CC - 1:
                        drain(kq_eng, kT[h][:, 512 * jj:512 * jj + 512],
                              ps[:, 0:512])
                return run

            def q_step(qb):
                def run():
                    ps = pj_pool.tile([128, 512], fp32, tag="po",
                                      name=f"psq{h}_{qb}")
                    for cc in range(CC):
                        nc.tensor.matmul(
                            ps[:, 0:512],
                            lhsT=wq4_sb[:, cc * 1024 + 128 * h: cc * 1024 + 128 * h + 128],
                            rhs=tgt_sb[:, cc * NQ + 512 * qb: cc * NQ + 512 * qb + 512],
                            start=(cc == 0), stop=(cc == CC - 1),
                            tile_position=(0, 0))
                    drain(kq_eng, qT[h][:, 512 * qb:512 * qb + 512],
                          ps[:, 0:512])
                return run

            def k_full(jj):
                box = []
                halves = [k_step(jj, cc, box) for cc in range(CC)]

                def run():
                    for st in halves:
                        st()
                return run

            for jj in range(2):
                steps.append(k_full(jj))
            for qb in range(NQ // 512):
                steps.append(q_step(qb))
            return steps

        def kqproj(h):
            for st in kqproj_steps(h):
                st()

        strips = (0, 1, 2, 3)
        po_tiles = {}
        merges = []

        def merge_pop():
            """Fold the pending unit's B half into its A half: one DVE
            tensor_tensor (PSUM A + hopped-B SBUF -> xm fp32, incl. the sums
            row), then DMA the merged sums into su.  Releases the po bank."""
            h, qb, po, bhop = merges.pop(0)
            nc.vector.tensor_add(xm[h][:, 512 * qb:512 * qb + 512],
                                 po[0:33, 0:512], bhop[0:33, :])
            nc.sync.dma_start(su[h][8 * qb:8 * qb + 8, 0:64],
                              xm[h][32:33, 512 * qb:512 * qb + 512])
            del po_tiles[(h, qb)]

        def attnv(h, qb, m, pbs):
            """attn@v quad for (unit, round m), col-tiled 2x: strips 0/2
            accumulate at partitions 0-32 (A), strips 1/3 at 64-96 (B) of the
            same bank-columns, so the two chains stream concurrently.  po is
            allocated lazily at the unit's first (lagged) quad."""
            if (h, qb) not in po_tiles:
                po_tiles[(h, qb)] = po_pool.tile(
                    [128, 512], fp32, tag="po", name=f"po{h}_{qb}")
            po = po_tiles[(h, qb)]
            for gi, g in enumerate(strips):
                kc = 16 * (m // 4) + 4 * g + (m % 4)
                co = 64 * (gi % 2)
                nc.tensor.matmul(
                    po[co:co + 33, 0:512],
                    lhsT=v_sb[:, 1056 * h + 33 * kc: 1056 * h + 33 * kc + 33],
                    rhs=pbs[gi // 2][:, 512 * (gi % 2):512 * (gi % 2) + 512],
                    start=(m == 0 and gi <= 1), stop=(m == 7 and gi >= 2),
                    skip_group_check=True,
                    tile_position=(0, co))
            if m == 7:
                # B half drains on ACT and hops to partitions 0-32 by DMA;
                # the A half stays in PSUM until merge() adds them in a
                # single DVE tensor_tensor (PSUM + SBUF -> xm), two rounds
                # later.  gpsimd runs ONLY partition_broadcast (mixing in
                # other gpsimd ucode would force 6us IRAM reloads per head).
                stgb = xb_pool.tile([97, 512], fp32, tag="stgb",
                                    name=f"sb{h}_{qb}")
                bhop = xb_pool.tile([33, 512], fp32, tag="bhop",
                                    name=f"bh{h}_{qb}")
                nc.vector.tensor_copy(stgb[64:97, :], po[64:97, 0:512])
                nc.sync.dma_start(bhop[0:33, :], stgb[64:97, :])
                merges.append((h, qb, po, bhop))

        def score_exp(h, qb, ri, feed):
            """scores quad + exp pair for (unit, round ri); feed maps round
            index -> closure (pinned so deps are emitted in order)."""
            m = ri
            st = feed.pop(ri, None)
            if st is not None:
                st()
            pss = [ps_pool.tile([128, 1024], fp32, tag="ps",
                                name=f"ps{h}_{qb}_{ri}_{i}")
                   for i in range(2)]
            for gi, g in enumerate(strips):
                nc.tensor.matmul(
                    pss[gi // 2][:, 512 * (gi % 2):512 * (gi % 2) + 512],
                    lhsT=kT[h][32 * g:32 * g + 32, 128 * m:128 * m + 128],
                    rhs=qT[h][32 * g:32 * g + 32, 512 * qb:512 * qb + 512],
                    start=True, stop=True,
                    tile_position=(32 * g, 0))
            pbs = []
            for i in range(2):
                p_sb = p_pool.tile([128, 1024], bf16, tag="p",
                                   name=f"p{h}_{qb}_{ri}_{i}")
                if (ri, i) in CFG["dve_tiles"]:
                    nc.vector.tensor_scalar(
                        p_sb[:].bitcast(i16), pss[i][:, 0:1024],
                        EXPA, EXPB, Mult, Add)
                else:
                    nc.scalar.activation(p_sb[:], pss[i][:, 0:1024], Exp)
                pbs.append(p_sb)
            # vproj for this round's chunks emits after the exps so the exp
            # stream starts as early as possible during the first unit
            if h == 0 and qb == 0:
                for g in strips:
                    vproj(16 * (m // 4) + 4 * g + (m % 4))
            if ri == 3 and merges:
                merge_pop()
            return pbs

        def norm_steps(h):
            """Normalization closures for head h, to be fed under later
            units.  Steady-state heads broadcast 1/denominator by doubling
            DMAs and multiply on gpsimd (exp engines stay free); the last two
            heads use a K=1 PE-matmul broadcast + DVE multiply, which has a
            much shorter serial chain, to compress the kernel tail."""
            tail = h >= 6
            rrow = sm_pool.tile([1, NQ], bf16 if tail else fp32, tag="rrow",
                                name=f"rr{h}")

            def stack():
                t, r = h // 4, 32 * (h % 4)
                nc.sync.dma_start(xfs[t][r:r + 32, :], xf[h][:])

            if not tail:
                rbs = sm_pool.tile([32, NQ], fp32, tag="rbs", name=f"rb{h}")

                def s_recip():
                    nc.vector.reciprocal_approx_fast(ru[h][:], su[h][:])
                    nc.sync.dma_start(rrow[:], ru[h][:])
                    nc.sync.dma_start(rbs[0:1, :], rrow[:])
                    for w in (1, 2, 4, 8, 16):
                        nc.sync.dma_start(rbs[w:2 * w, :], rbs[0:w, :])

                def s_mul():
                    nc.gpsimd.tensor_mul(xf[h][:], xm[h][0:32, :], rbs[:])
                    stack()

                return [s_recip, s_mul, lambda: None]

            def s_recip():
                nc.vector.reciprocal_approx_fast(ru[h][:], su[h][:])
                nc.vector.tensor_copy(rub[h][:], ru[h][:])
                nc.sync.dma_start(rrow[:], rub[h][:])

            def s_mul(q2):
                rps = pj_pool.tile([128, 512], fp32, tag="po",
                                   name=f"rb{h}_{q2}")
                nc.tensor.matmul(
                    rps[0:32, 0:512], lhsT=ones1_sb[0:1, 0:32],
                    rhs=rrow[0:1, 512 * q2:512 * q2 + 512],
                    start=True, stop=True, tile_position=(0, 0))
                nc.vector.tensor_mul(
                    xf[h][:, 512 * q2:512 * q2 + 512],
                    xm[h][0:32, 512 * q2:512 * q2 + 512],
                    rps[0:32, 0:512])
                if q2 == 1:
                    stack()

            return [s_recip, lambda: s_mul(0), lambda: s_mul(1)]

        def out_proj():
            for dc in range(CC):
                for qb in range(NQ // 512):
                    ps = pj_pool.tile([128, 512], fp32, tag="po",
                                      name=f"py{dc}_{qb}")
                    # tgt chunks first, xfs last: the first MMs can run
                    # while the final heads' normalization still completes
                    rhss = [(cc, tgt_sb[:, cc * NQ + 512 * qb:
                                        cc * NQ + 512 * qb + 512])
                            for cc in range(CC)]
                    rhss += [(cc, xfs[cc][:, 512 * qb:512 * qb + 512])
                             for cc in range(CC)]
                    for n_mm, (cc, rhs) in enumerate(rhss):
                        nc.tensor.matmul(
                            ps[:, 0:512],
                            lhsT=wo_sb[:, cc * C + 128 * dc: cc * C + 128 * dc + 128],
                            rhs=rhs,
                            start=(n_mm == 0), stop=(n_mm == 3),
                            tile_position=(0, 0))
                    yt = xb_pool.tile([128, 512], fp32, tag="yt",
                                      name=f"yt{dc}_{qb}")
                    nc.vector.tensor_copy(yt[:], ps[:, 0:512])
                    nc.sync.dma_start(
                        y_d.ap()[128 * dc:128 * (dc + 1),
                                 512 * qb:512 * qb + 512], yt[:])

        # ---- emission: one flat global round stream across all 16 units;
        # attn@v lags scores/exp by ATTNV_LAG rounds so it never waits on
        # exp and unit boundaries don't drain the pipeline -------------------
        ATTNV_LAG = 3
        kqproj(0)
        units = [(h, qb) for h in range(HEADS) for qb in range(NQ // 512)]
        # feed maps round index -> closure.  kqproj steps early; norm steps
        # pinned at round >= 2 so they trail the previous unit's drain/su
        # gather, which the lag-2 attn@v emits at rounds 0-1 of this unit.
        feeds = {}
        for h in range(HEADS):
            steps = kqproj_steps(h + 1) if h + 1 < HEADS else []
            nsteps = norm_steps(h - 1) if h >= 1 else []
            feeds[(h, 0)] = {}
            feeds[(h, 1)] = {}
            if steps:
                feeds[(h, 0)][0] = steps[0]
                feeds[(h, 0)][1] = steps[1]
                feeds[(h, 1)][0] = steps[2]
                feeds[(h, 1)][1] = steps[3]
            if nsteps:
                feeds[(h, 0)][6] = nsteps[0]
                feeds[(h, 1)][5] = nsteps[1]
                feeds[(h, 1)][7] = nsteps[2]
        pend = []
        for ui, (h, qb) in enumerate(units):
            feed = feeds[(h, qb)]
            # unit 0 runs at a deeper lag so its vproj PE work doesn't
            # throttle the exp stream at startup; drained 2 quads/round after
            lag = 6 if ui == 0 else ATTNV_LAG
            for ri in range(8):
                pbs = score_exp(h, qb, ri, feed)
                pend.append((h, qb, ri, pbs))
                for _ in range(min(max(len(pend) - lag, 0), 2)):
                    attnv(*pend.pop(0))
            assert not feed
        for args in pend:
            attnv(*args)
        while merges:
            merge_pop()
        for st in norm_steps(HEADS - 1):
            st()
        out_proj()

    nc.compile()
    return nc


def _prep_core_inputs(core, tgt, src, Wq, Wk, Wv, Wo):
    b, qoff = core // 4, NQ * (core % 4)
    srcT = src[b].reshape(C, N)
    tgtT = tgt[b].reshape(C, N)[:, qoff:qoff + NQ]
    scale = 1.0 / np.sqrt(np.float32(D))
    wqT = (Wq * scale).T.astype(BF16)
    wq4 = np.empty((C, HEADS * 128), dtype=BF16)
    for h in range(HEADS):
        wq4[:, 128 * h:128 * (h + 1)] = np.tile(wqT[:, 32 * h:32 * h + 32],
                                                (1, 4))
    return {
        "src_bf": np.ascontiguousarray(srcT).astype(BF16),
        "tgt_bf": np.ascontiguousarray(tgtT).astype(BF16),
        "wq4": wq4,
        "wkT": np.ascontiguousarray(Wk.T).astype(BF16),
        "wvT": np.ascontiguousarray(Wv.T).astype(BF16),
        "woT": np.ascontiguousarray(Wo.T).astype(BF16),
    }


def kernel(tgt, src, Wq, Wk, Wv, Wo, _want_results=False):
    from concourse.bass_utils import run_bass_kernel_spmd

    tgt = np.asarray(tgt, dtype=np.float32)
    src = np.asarray(src, dtype=np.float32)
    Wq = np.asarray(Wq, dtype=np.float32)
    Wk = np.asarray(Wk, dtype=np.float32)
    Wv = np.asarray(Wv, dtype=np.float32)
    Wo = np.asarray(Wo, dtype=np.float32)

    if "nc" not in _cached:
        _cached["nc"] = _build_nc()
    nc = _cached["nc"]

    in_maps = [_prep_core_inputs(c, tgt, src, Wq, Wk, Wv, Wo)
               for c in range(NCORES)]
    res = run_bass_kernel_spmd(nc, in_maps, core_ids=list(range(NCORES)))

    out = np.empty((B, N, C), dtype=np.float32)
    for c in range(NCORES):
        b, qoff = c // 4, NQ * (c % 4)
        out[b, qoff:qoff + NQ, :] = res.results[c]["yT"].T
    if _want_results:
        return out, res
    return out
